# revision 13
# baseline (speedup 1.0000x reference)
"""Distributed multi-head attention kernel for 8 TRN2 NeuronCores.

Sharding: 8-way head parallel (2 heads per core), batches looped on-core.
Each core: QKV projection for its 2 heads over both batches, per-head
attention (softmax without max-subtraction — logits are small; denominators
come from a ones-column appended to V so they fall out of the attn@V
matmul), then ONE AllToAll across all 8 cores exchanges row-blocks for
head-blocks: block s = (batch s//4, rows-block s%4).  Core c ends up with
all 1024 inner dims for (batch c//4, rows [(c%4)*512, ...)) and runs the
full output projection + bias on that slice.  The A2A moves 1MB of bf16
per core instead of all-reducing 8.4MB of fp32.

x and the weight matrices are cast to bf16 on the host (compute precision
is bf16 anyway) so the kernel needs no on-chip staging casts and can use
the XBAR DMA transpose to build x^T directly.

The per-core output is the TRANSPOSED final slice [1024, 512] (PSUM-major
writes stay contiguous); the host transposes during assembly.
"""
import numpy as np

import concourse.bass as bass
import concourse.mybir as mybir
from concourse import bacc
import concourse.tile as tile
from concourse.bass_utils import run_bass_kernel_spmd

# problem constants (hardcoded; kernel.py must be self-contained)
B, N, DIM = 2, 2048, 1024
H, DH = 16, 64
INNER = H * DH            # 1024
SCALE = DIM ** -0.5       # 1/32  (module scales by dim**-0.5, not dim_head)
NCORES = 8
HPC = H // NCORES         # 2 heads per core
SH = HPC * DH             # 128 inner cols per core
ROWS = N // 4             # 512 output rows per core
P = 128
KO = DIM // P             # 8 contraction chunks
JC = N // P               # 16 row chunks
IB = 512                  # query block size
NIB = N // IB             # 4 query blocks
FP32 = mybir.dt.float32
BF16 = mybir.dt.bfloat16

REPLICA_GROUPS = [[0, 1, 2, 3, 4, 5, 6, 7]]

_NC_CACHE = {}

# set by the last kernel() call when BASS_KERNEL_TRACE=1 (for test.py)
LAST_RESULTS = None


def _build():
    nc = bacc.Bacc(num_devices=NCORES)

    x_ext = nc.declare_dram_parameter("x", [B * N, DIM], BF16, isOutput=False)
    wq_ext = nc.declare_dram_parameter("wq", [DIM, SH], BF16, isOutput=False)
    wk_ext = nc.declare_dram_parameter("wk", [DIM, SH], BF16, isOutput=False)
    wv_ext = nc.declare_dram_parameter("wv", [DIM, SH], BF16, isOutput=False)
    wo_ext = nc.declare_dram_parameter("wo", [DIM, DIM], BF16, isOutput=False)
    bo_ext = nc.declare_dram_parameter("bo", [DIM], FP32, isOutput=False)
    out_ext = nc.declare_dram_parameter("out", [DIM, ROWS], FP32, isOutput=True)

    with tile.TileContext(nc) as tc:
        with (
            tc.tile_pool(name="consts", bufs=1) as consts,
            tc.tile_pool(name="stage", bufs=3) as stage,
            tc.tile_pool(name="xt_pool", bufs=2) as xt_pool,
            tc.tile_pool(name="dram", bufs=1, space="DRAM") as dram,
        ):
            # ---- weights: already bf16, direct strided loads ----
            wq_sb = consts.tile([P, KO, SH], BF16)
            wk_sb = consts.tile([P, KO, SH], BF16)
            wv_sb = consts.tile([P, KO, SH], BF16)
            wo_sb = consts.tile([P, KO, DIM], BF16)
            for w_ext, w_sb in (
                (wq_ext, wq_sb),
                (wk_ext, wk_sb),
                (wv_ext, wv_sb),
                (wo_ext, wo_sb),
            ):
                nc.sync.dma_start(
                    w_sb, w_ext.rearrange("(ko kp) c -> kp ko c", kp=P)
                )
            ident_bf = consts.tile([P, P], BF16)
            from concourse.masks import make_identity
            make_identity(nc, ident_bf)
            bias_sb = consts.tile([P, KO], FP32)
            nc.sync.dma_start(
                bias_sb, bo_ext.rearrange("(co cp) -> cp co", cp=P)
            )

            # shared PSUM pools for the whole kernel (no phase walls):
            # 3x 2-bank "st" slots + 2x 1-bank "po" slots = 8 banks
            _st_cm = tc.tile_pool(name="st_psum", bufs=3, space="PSUM")
            st_psum = _st_cm.__enter__()
            _o_cm = tc.tile_pool(name="o_psum", bufs=2, space="PSUM")
            o_psum = _o_cm.__enter__()

            # persistent per-batch-indexed tensors
            qT = consts.tile([P, B, N], BF16)    # [h*64+d, b, i]
            kT = consts.tile([P, B, N], BF16)
            v_aug = consts.tile([P, B, JC, HPC, DH + 1], BF16)
            nc.vector.memset(v_aug[:, :, :, :, DH : DH + 1], 1.0)
            out_rows = consts.tile([P, B, JC, SH], BF16)

            for b in range(B):
                # ---- x[b]^T via XBAR DMA transpose (bf16), nb-chunked so
                # qkv matmuls can start as soon as the first column block lands
                xT = xt_pool.tile([P, KO, N], BF16, tag="xT", name="xT")
                qdma = [nc.sync]
                di = 0
                for nb in range(NIB):
                    for ko in range(KO):
                        qdma[di % len(qdma)].dma_start_transpose(
                            xT[:, ko, nb * IB : (nb + 1) * IB],
                            x_ext[
                                b * N + nb * IB : b * N + (nb + 1) * IB,
                                ko * P : (ko + 1) * P,
                            ],
                        )
                        di += 1

                # ---- QKV projection for batch b (shared PSUM slots) ----
                for w_sb, dstT in ((wq_sb, qT), (wk_sb, kT)):
                    for nb in range(NIB):
                        ps2 = st_psum.tile(
                            [P, 2, IB], FP32, tag="st", name="qk_ps"
                        )
                        ps = ps2[:, 0, :]
                        for ko in range(KO):
                            nc.tensor.matmul(
                                ps,
                                w_sb[:, ko, :],
                                xT[:, ko, nb * IB : (nb + 1) * IB],
                                start=(ko == 0),
                                stop=(ko == KO - 1),
                            )
                        nc.vector.tensor_copy(
                            dstT[:, b, nb * IB : (nb + 1) * IB], ps
                        )
                for mt in range(JC):
                    psv2 = st_psum.tile([P, 2, IB], FP32, tag="st", name="v_ps")
                    ps_v = psv2[:, 0, :SH]
                    for ko in range(KO):
                        nc.tensor.matmul(
                            ps_v,
                            xT[:, ko, mt * P : (mt + 1) * P],
                            wv_sb[:, ko, :],
                            start=(ko == 0),
                            stop=(ko == KO - 1),
                        )
                    nc.vector.tensor_copy(
                        v_aug[:, b, mt, :, 0:DH],
                        ps_v.rearrange("p (h d) -> p h d", d=DH),
                    )

            # ---- attention + progressive A2A input staging ----
            # ST[j, i] = k @ q.T; exp(SCALE*ST) is safe without max-subtraction
            # (logits ~N(0, 0.25^2)).  Loop order b -> ib -> h so row-block
            # (b, ib) is final after the h loop and its A2A send DMA can fire
            # while later blocks still compute.
            a2a_full_in = dram.tile([NCORES, NIB, P, SH], BF16, name="a2a_full_in")
            a2a_full_out = dram.tile([NCORES, ROWS, SH], BF16, name="a2a_full_out")
            attnT = xt_pool.tile([P, KO, N], BF16, tag="xT", name="attnT")[:, :, :ROWS]
            with (
                tc.tile_pool(name="pt_pool", bufs=2) as pt_pool,
                tc.tile_pool(name="nrm", bufs=4) as nrm,
            ):
                for h in range(HPC):
                    for b in range(B):
                        for ib in range(NIB):
                            po = h * DH
                            ptile = pt_pool.tile(
                                [P, JC, IB], BF16, tag="pt", name="ptile"
                            )
                            for jg in range(JC // 2):
                                ps_st = st_psum.tile(
                                    [P, 2, IB], FP32, tag="st", name="st_ps"
                                )
                                for u in range(2):
                                    jc = jg * 2 + u
                                    nc.tensor.matmul(
                                        ps_st[:, u, :],
                                        kT[po : po + DH, b, jc * P : (jc + 1) * P],
                                        qT[po : po + DH, b, ib * IB : (ib + 1) * IB],
                                        start=True,
                                        stop=True,
                                    )
                                nc.scalar.activation(
                                    ptile[:, jg * 2 : (jg + 1) * 2, :],
                                    ps_st,
                                    mybir.ActivationFunctionType.Exp,
                                    scale=SCALE,
                                )
                            for isub in range(IB // P):
                                ic = ib * (IB // P) + isub
                                ps_o = o_psum.tile(
                                    [P, DH + 1], FP32, tag="po", name="o_ps"
                                )
                                for jc in range(JC):
                                    nc.tensor.matmul(
                                        ps_o,
                                        ptile[:, jc, isub * P : (isub + 1) * P],
                                        v_aug[:, b, jc, h, :],
                                        start=(jc == 0),
                                        stop=(jc == JC - 1),
                                    )
                                recip = nrm.tile(
                                    [P, 1], FP32, tag="recip", name="recip"
                                )
                                nc.vector.reciprocal(recip, ps_o[:, DH : DH + 1])
                                nc.vector.tensor_scalar_mul(
                                    out_rows[:, b, ic, po : po + DH],
                                    ps_o[:, 0:DH],
                                    recip,
                                )
                            if h == HPC - 1:
                                # both heads of block (b, ib) done -> stage it
                                s = b * NIB + ib
                                nc.sync.dma_start(
                                    a2a_full_in[s].rearrange("ic p c -> p ic c"),
                                    out_rows[
                                        :,
                                        b,
                                        ib * (IB // P) : (ib + 1) * (IB // P),
                                        :,
                                    ],
                                )
            nc.gpsimd.collective_compute(
                "AllToAll",
                mybir.AluOpType.bypass,
                replica_groups=REPLICA_GROUPS,
                ins=[a2a_full_in.opt()],
                outs=[a2a_full_out.opt()],
            )
            for i in range(NCORES):
                nc.sync.dma_start_transpose(attnT[:, i, :], a2a_full_out[i])

            # ---- output projection ----
            if True:
                for cc in range(KO):
                    psf2 = st_psum.tile([P, 2, IB], FP32, tag="st", name="f_ps")
                    ps_f = psf2[:, 0, :ROWS]
                    for ko in range(KO):
                        nc.tensor.matmul(
                            ps_f,
                            wo_sb[:, ko, cc * P : (cc + 1) * P],
                            attnT[:, ko, :],
                            start=(ko == 0),
                            stop=(ko == KO - 1),
                        )
                    of = stage.tile([P, ROWS], FP32, tag="of", name="of")
                    nc.vector.tensor_scalar_add(of, ps_f, bias_sb[:, cc : cc + 1])
                    nc.sync.dma_start(out_ext[cc * P : (cc + 1) * P, :], of)

            _o_cm.__exit__(None, None, None)
            _st_cm.__exit__(None, None, None)

    nc.finalize()
    return nc


def _get_nc():
    if "nc" not in _NC_CACHE:
        _NC_CACHE["nc"] = _build()
    return _NC_CACHE["nc"]


def kernel(**inputs) -> np.ndarray:
    import os

    import ml_dtypes

    global LAST_RESULTS

    bf16 = ml_dtypes.bfloat16
    x = np.asarray(inputs["x"], dtype=np.float32)
    W_qkv = np.asarray(inputs["W_qkv"], dtype=np.float32)
    W_out = np.asarray(inputs["W_out"], dtype=np.float32)
    b_out = np.ascontiguousarray(np.asarray(inputs["b_out"], dtype=np.float32))

    x_bf = np.ascontiguousarray(x.reshape(B * N, DIM).astype(bf16))
    wo_bf = np.ascontiguousarray(W_out.astype(bf16))
    wqkv_bf = W_qkv.astype(bf16)

    nc = _get_nc()

    in_maps = []
    for c in range(NCORES):
        in_maps.append(
            {
                "x": x_bf,
                "wq": np.ascontiguousarray(
                    wqkv_bf[:, 0 * INNER + c * SH : 0 * INNER + (c + 1) * SH]
                ),
                "wk": np.ascontiguousarray(
                    wqkv_bf[:, 1 * INNER + c * SH : 1 * INNER + (c + 1) * SH]
                ),
                "wv": np.ascontiguousarray(
                    wqkv_bf[:, 2 * INNER + c * SH : 2 * INNER + (c + 1) * SH]
                ),
                "wo": wo_bf,
                "bo": b_out,
            }
        )

    trace = os.environ.get("BASS_KERNEL_TRACE", "0") == "1"
    res = run_bass_kernel_spmd(
        nc, in_maps, core_ids=list(range(NCORES)), trace=trace
    )
    LAST_RESULTS = res

    y = np.empty((B, N, DIM), dtype=np.float32)
    for c in range(NCORES):
        b, r = c // 4, c % 4
        y[b, r * ROWS : (r + 1) * ROWS, :] = res.results[c]["out"].T
    return y


# revision 14
# speedup vs baseline: 1.0494x; 1.0494x over previous
"""Distributed multi-head attention kernel for 8 TRN2 NeuronCores.

Sharding: 8-way head parallel (2 heads per core), batches looped on-core.
Each core: QKV projection for its 2 heads over both batches, per-head
attention (softmax without max-subtraction — logits are small; denominators
come from a ones-column appended to V so they fall out of the attn@V
matmul), then ONE AllToAll across all 8 cores exchanges row-blocks for
head-blocks: block s = (batch s//4, rows-block s%4).  Core c ends up with
all 1024 inner dims for (batch c//4, rows [(c%4)*512, ...)) and runs the
full output projection + bias on that slice.  The A2A moves 1MB of bf16
per core instead of all-reducing 8.4MB of fp32.

x and the weight matrices are cast to bf16 on the host (compute precision
is bf16 anyway) so the kernel needs no on-chip staging casts and can use
the XBAR DMA transpose to build x^T directly.

The per-core output is the TRANSPOSED final slice [1024, 512] (PSUM-major
writes stay contiguous); the host transposes during assembly.
"""
import numpy as np

import concourse.bass as bass
import concourse.mybir as mybir
from concourse import bacc
import concourse.tile as tile
from concourse.bass_utils import run_bass_kernel_spmd

# problem constants (hardcoded; kernel.py must be self-contained)
B, N, DIM = 2, 2048, 1024
H, DH = 16, 64
INNER = H * DH            # 1024
SCALE = DIM ** -0.5       # 1/32  (module scales by dim**-0.5, not dim_head)
NCORES = 8
HPC = H // NCORES         # 2 heads per core
SH = HPC * DH             # 128 inner cols per core
ROWS = N // 4             # 512 output rows per core
P = 128
KO = DIM // P             # 8 contraction chunks
JC = N // P               # 16 row chunks
IB = 512                  # query block size
NIB = N // IB             # 4 query blocks
FP32 = mybir.dt.float32
BF16 = mybir.dt.bfloat16

REPLICA_GROUPS = [[0, 1, 2, 3, 4, 5, 6, 7]]

_NC_CACHE = {}

# set by the last kernel() call when BASS_KERNEL_TRACE=1 (for test.py)
LAST_RESULTS = None


def _build():
    nc = bacc.Bacc(num_devices=NCORES)

    x_ext = nc.declare_dram_parameter("x", [B * N, DIM], BF16, isOutput=False)
    wq_ext = nc.declare_dram_parameter("wq", [DIM, SH], BF16, isOutput=False)
    wk_ext = nc.declare_dram_parameter("wk", [DIM, SH], BF16, isOutput=False)
    wv_ext = nc.declare_dram_parameter("wv", [DIM, SH], BF16, isOutput=False)
    wo_ext = nc.declare_dram_parameter("wo", [DIM, DIM], BF16, isOutput=False)
    bo_ext = nc.declare_dram_parameter("bo", [DIM], FP32, isOutput=False)
    out_ext = nc.declare_dram_parameter("out", [DIM, ROWS], FP32, isOutput=True)

    with tile.TileContext(nc) as tc:
        with (
            tc.tile_pool(name="consts", bufs=1) as consts,
            tc.tile_pool(name="stage", bufs=3) as stage,
            tc.tile_pool(name="xt_pool", bufs=2) as xt_pool,
            tc.tile_pool(name="dram", bufs=1, space="DRAM") as dram,
        ):
            # ---- weights: already bf16, direct strided loads ----
            wq_sb = consts.tile([P, KO, SH], BF16)
            wk_sb = consts.tile([P, KO, SH], BF16)
            wv_sb = consts.tile([P, KO, SH], BF16)
            wo_sb = consts.tile([P, KO, DIM], BF16)
            for w_ext, w_sb in (
                (wq_ext, wq_sb),
                (wk_ext, wk_sb),
                (wv_ext, wv_sb),
                (wo_ext, wo_sb),
            ):
                nc.sync.dma_start(
                    w_sb, w_ext.rearrange("(ko kp) c -> kp ko c", kp=P)
                )
            ident_bf = consts.tile([P, P], BF16)
            from concourse.masks import make_identity
            make_identity(nc, ident_bf)
            bias_sb = consts.tile([P, KO], FP32)
            nc.sync.dma_start(
                bias_sb, bo_ext.rearrange("(co cp) -> cp co", cp=P)
            )

            # shared PSUM pools for the whole kernel (no phase walls):
            # 3x 2-bank "st" slots + 2x 1-bank "po" slots = 8 banks
            _st_cm = tc.tile_pool(name="st_psum", bufs=3, space="PSUM")
            st_psum = _st_cm.__enter__()
            _o_cm = tc.tile_pool(name="o_psum", bufs=2, space="PSUM")
            o_psum = _o_cm.__enter__()

            # persistent per-batch-indexed tensors
            qT = consts.tile([P, B, N], BF16)    # [h*64+d, b, i]
            kT = consts.tile([P, B, N], BF16)
            v_aug = consts.tile([P, B, JC, HPC, DH + 1], BF16)
            nc.vector.memset(v_aug[:, :, :, :, DH : DH + 1], 1.0)
            out_rows = consts.tile([P, B, JC, SH], BF16)

            for b in range(B):
                # ---- x[b]^T via PE transposes (no XBAR: loads spread across
                # DMA queues and the PE is idle during the ramp anyway) ----
                xT = xt_pool.tile([P, KO, N], BF16, tag="xT", name="xT")
                for mt in range(JC):
                    xst = stage.tile([P, DIM], BF16, tag="xst", name="xst")
                    nc.sync.dma_start(
                        xst, x_ext[b * N + mt * P : b * N + (mt + 1) * P, :]
                    )
                    for kg in range(2):
                        tp_ps = o_psum.tile(
                            [P, 4, P], BF16, tag="po", name="tp_ps"
                        )
                        for q in range(4):
                            ko = kg * 4 + q
                            nc.tensor.transpose(
                                tp_ps[:, q, :],
                                xst[:, ko * P : (ko + 1) * P],
                                ident_bf,
                            )
                        nc.vector.tensor_copy(
                            xT[:, kg * 4 : (kg + 1) * 4, mt * P : (mt + 1) * P],
                            tp_ps,
                        )

                # ---- QKV projection for batch b (shared PSUM slots) ----
                for w_sb, dstT in ((wq_sb, qT), (wk_sb, kT)):
                    for nb in range(NIB):
                        ps2 = st_psum.tile(
                            [P, 2, IB], FP32, tag="st", name="qk_ps"
                        )
                        ps = ps2[:, 0, :]
                        for ko in range(KO):
                            nc.tensor.matmul(
                                ps,
                                w_sb[:, ko, :],
                                xT[:, ko, nb * IB : (nb + 1) * IB],
                                start=(ko == 0),
                                stop=(ko == KO - 1),
                            )
                        nc.vector.tensor_copy(
                            dstT[:, b, nb * IB : (nb + 1) * IB], ps
                        )
                for mt in range(JC):
                    psv2 = st_psum.tile([P, 2, IB], FP32, tag="st", name="v_ps")
                    ps_v = psv2[:, 0, :SH]
                    for ko in range(KO):
                        nc.tensor.matmul(
                            ps_v,
                            xT[:, ko, mt * P : (mt + 1) * P],
                            wv_sb[:, ko, :],
                            start=(ko == 0),
                            stop=(ko == KO - 1),
                        )
                    nc.vector.tensor_copy(
                        v_aug[:, b, mt, :, 0:DH],
                        ps_v.rearrange("p (h d) -> p h d", d=DH),
                    )

            # ---- attention + progressive A2A input staging ----
            # ST[j, i] = k @ q.T; exp(SCALE*ST) is safe without max-subtraction
            # (logits ~N(0, 0.25^2)).  Loop order b -> ib -> h so row-block
            # (b, ib) is final after the h loop and its A2A send DMA can fire
            # while later blocks still compute.
            a2a_in0 = dram.tile([NCORES, NIB, P, DH], BF16, name="a2a_in0")
            a2a_in1 = dram.tile([NCORES, NIB, P, DH], BF16, name="a2a_in1")
            a2a_out0 = dram.tile([NCORES, ROWS, DH], BF16, name="a2a_out0")
            a2a_out1 = dram.tile([NCORES, ROWS, DH], BF16, name="a2a_out1")
            a2a_ins = [a2a_in0, a2a_in1]
            a2a_outs = [a2a_out0, a2a_out1]
            attnT = xt_pool.tile([P, KO, N], BF16, tag="xT", name="attnT")[:, :, :ROWS]
            with (
                tc.tile_pool(name="pt_pool", bufs=2) as pt_pool,
                tc.tile_pool(name="nrm", bufs=4) as nrm,
            ):
                for h in range(HPC):
                    for b in range(B):
                        for ib in range(NIB):
                            po = h * DH
                            ptile = pt_pool.tile(
                                [P, JC, IB], BF16, tag="pt", name="ptile"
                            )
                            for jg in range(JC // 2):
                                ps_st = st_psum.tile(
                                    [P, 2, IB], FP32, tag="st", name="st_ps"
                                )
                                for u in range(2):
                                    jc = jg * 2 + u
                                    nc.tensor.matmul(
                                        ps_st[:, u, :],
                                        kT[po : po + DH, b, jc * P : (jc + 1) * P],
                                        qT[po : po + DH, b, ib * IB : (ib + 1) * IB],
                                        start=True,
                                        stop=True,
                                    )
                                nc.scalar.activation(
                                    ptile[:, jg * 2 : (jg + 1) * 2, :],
                                    ps_st,
                                    mybir.ActivationFunctionType.Exp,
                                    scale=SCALE,
                                )
                            for isub in range(IB // P):
                                ic = ib * (IB // P) + isub
                                ps_o = o_psum.tile(
                                    [P, DH + 1], FP32, tag="po", name="o_ps"
                                )
                                for jc in range(JC):
                                    nc.tensor.matmul(
                                        ps_o,
                                        ptile[:, jc, isub * P : (isub + 1) * P],
                                        v_aug[:, b, jc, h, :],
                                        start=(jc == 0),
                                        stop=(jc == JC - 1),
                                    )
                                recip = nrm.tile(
                                    [P, 1], FP32, tag="recip", name="recip"
                                )
                                nc.vector.reciprocal(recip, ps_o[:, DH : DH + 1])
                                nc.vector.tensor_scalar_mul(
                                    out_rows[:, b, ic, po : po + DH],
                                    ps_o[:, 0:DH],
                                    recip,
                                )
                            # block (h, b, ib) complete -> stage its A2A input
                            s = b * NIB + ib
                            nc.sync.dma_start(
                                a2a_ins[h][s].rearrange("ic p c -> p ic c"),
                                out_rows[
                                    :,
                                    b,
                                    ib * (IB // P) : (ib + 1) * (IB // P),
                                    po : po + DH,
                                ],
                            )
                    # all of head h staged on every core -> exchange + receive;
                    # the h=0 collective overlaps head-1 compute
                    nc.gpsimd.collective_compute(
                        "AllToAll",
                        mybir.AluOpType.bypass,
                        replica_groups=REPLICA_GROUPS,
                        ins=[a2a_ins[h].opt()],
                        outs=[a2a_outs[h].opt()],
                    )
                    for i in range(NCORES):
                        rstage = pt_pool.tile(
                            [P, NIB, DH], BF16, tag="rstage", name="rstage"
                        )
                        nc.sync.dma_start(
                            rstage,
                            a2a_outs[h][i].rearrange("(ic p) c -> p ic c", p=P),
                        )
                        rps = o_psum.tile([DH, NIB, P], BF16, tag="po", name="r_ps")
                        for q in range(NIB):
                            nc.tensor.transpose(
                                rps[:, q, :], rstage[:, q, :], ident_bf
                            )
                        nc.vector.tensor_copy(attnT[po : po + DH, i, :], rps)

            # ---- output projection ----
            if True:
                for cc in range(KO):
                    psf2 = st_psum.tile([P, 2, IB], FP32, tag="st", name="f_ps")
                    ps_f = psf2[:, 0, :ROWS]
                    for ko in range(KO):
                        nc.tensor.matmul(
                            ps_f,
                            wo_sb[:, ko, cc * P : (cc + 1) * P],
                            attnT[:, ko, :],
                            start=(ko == 0),
                            stop=(ko == KO - 1),
                        )
                    of = stage.tile([P, ROWS], FP32, tag="of", name="of")
                    nc.vector.tensor_scalar_add(of, ps_f, bias_sb[:, cc : cc + 1])
                    nc.sync.dma_start(out_ext[cc * P : (cc + 1) * P, :], of)

            _o_cm.__exit__(None, None, None)
            _st_cm.__exit__(None, None, None)

    nc.finalize()
    return nc


def _get_nc():
    if "nc" not in _NC_CACHE:
        _NC_CACHE["nc"] = _build()
    return _NC_CACHE["nc"]


def kernel(**inputs) -> np.ndarray:
    import os

    import ml_dtypes

    global LAST_RESULTS

    bf16 = ml_dtypes.bfloat16
    x = np.asarray(inputs["x"], dtype=np.float32)
    W_qkv = np.asarray(inputs["W_qkv"], dtype=np.float32)
    W_out = np.asarray(inputs["W_out"], dtype=np.float32)
    b_out = np.ascontiguousarray(np.asarray(inputs["b_out"], dtype=np.float32))

    x_bf = np.ascontiguousarray(x.reshape(B * N, DIM).astype(bf16))
    wo_bf = np.ascontiguousarray(W_out.astype(bf16))
    wqkv_bf = W_qkv.astype(bf16)

    nc = _get_nc()

    in_maps = []
    for c in range(NCORES):
        in_maps.append(
            {
                "x": x_bf,
                "wq": np.ascontiguousarray(
                    wqkv_bf[:, 0 * INNER + c * SH : 0 * INNER + (c + 1) * SH]
                ),
                "wk": np.ascontiguousarray(
                    wqkv_bf[:, 1 * INNER + c * SH : 1 * INNER + (c + 1) * SH]
                ),
                "wv": np.ascontiguousarray(
                    wqkv_bf[:, 2 * INNER + c * SH : 2 * INNER + (c + 1) * SH]
                ),
                "wo": wo_bf,
                "bo": b_out,
            }
        )

    trace = os.environ.get("BASS_KERNEL_TRACE", "0") == "1"
    res = run_bass_kernel_spmd(
        nc, in_maps, core_ids=list(range(NCORES)), trace=trace
    )
    LAST_RESULTS = res

    y = np.empty((B, N, DIM), dtype=np.float32)
    for c in range(NCORES):
        b, r = c // 4, c % 4
        y[b, r * ROWS : (r + 1) * ROWS, :] = res.results[c]["out"].T
    return y
